# revision 17
# baseline (speedup 1.0000x reference)
"""Cross-attention kernel for Trainium2, sharded over 8 NeuronCores.

Problem (hardcoded): b=4, n=m=2048, query_dim=context_dim=512,
heads=8, dim_head=64 (inner=512), f32 I/O.

Sharding: data-parallel over (batch, query-half): core c -> batch c//2,
query rows [(c%2)*1024, (c%2+1)*1024). Each core holds the full K/V
context for its batch, so there are no collectives and output shards
tile the full output exactly.

Layout strategy (all matmul inputs bf16, accumulation f32 in PSUM):
  - Host pre-transposes activations: pixelT [512c, 1024n], patchT [512c, 2048m].
  - Q^T [inner, n] and K^T [inner, m] computed with weights as stationary.
  - V kept natural [m, inner], stored per m-chunk as [128, 8 heads, 65]
    with a constant-1 column appended per head: the attention-output
    matmul then yields [65, n] per head where row 64 = sum(exp(scores)),
    giving the softmax denominator for free.
  - scores^T [m, n] per head via k=64 matmuls (kT stationary, qT moving);
    exp runs on ScalarE directly PSUM->SBUF(bf16) with scale=1/8 folded in.
  - attn-out^T [65, n] accumulates over 16 m-chunks in PSUM; normalized by
    reciprocal of row 64 (broadcast across partitions via gpsimd).
  - Final projection: outT (inner on partitions) stationary, Wo moving;
    bias added on the PSUM->SBUF copy.
"""

import numpy as np
import ml_dtypes

import concourse.bass as bass
import concourse.mybir as mybir
import concourse.tile as tile
from concourse import bacc
from concourse.bass_utils import run_bass_kernel_spmd

BF16 = mybir.dt.bfloat16
F32 = mybir.dt.float32

B, N, M = 4, 2048, 2048
CDIM, INNER = 512, 512
H, D = 8, 64
NSH = N // 2  # query rows per core
N_CORES = 8
SCALE = D ** -0.5

CC = CDIM // 128   # contraction chunks for projections (4)
IC = INNER // 128  # inner-dim chunks (4)
MT = M // 128      # m tiles (16)
NJ = NSH // 512    # n chunks of 512 (2)
NT = NSH // 128    # n tiles (8)
MJ = M // 512      # m chunks of 512 (4)

# Schraudolph exp-as-bit-trick constants for the DVE share of the softmax:
# bf16 bits of exp(s*SCALE) ~= s*EXPA + EXPB (computed f32, converted to i16,
# bitcast to bf16). Max rel err ~3.3% (floor) / 3.5% (round-to-nearest) on the
# attention weights, which washes out in the softmax-weighted average.
EXPA = 16 * float(np.log2(np.e))
EXPB = 16250.9


def build_nc() -> bass.Bass:
    nc = bacc.Bacc(None)

    pixelT = nc.dram_tensor("pixelT", [CDIM, NSH], BF16, kind="ExternalInput")
    patchT = nc.dram_tensor("patchT", [CDIM, M], BF16, kind="ExternalInput")
    wq = nc.dram_tensor("wq", [CDIM, INNER], BF16, kind="ExternalInput")
    wk = nc.dram_tensor("wk", [CDIM, INNER], BF16, kind="ExternalInput")
    wv = nc.dram_tensor("wv", [CDIM, INNER], BF16, kind="ExternalInput")
    wo = nc.dram_tensor("wo", [INNER, CDIM], BF16, kind="ExternalInput")
    bo = nc.dram_tensor("bo", [CDIM], BF16, kind="ExternalInput")
    out = nc.dram_tensor("out", [NSH, CDIM], F32, kind="ExternalOutput")

    with tile.TileContext(nc) as tc:
        with (
            tc.tile_pool(name="weights", bufs=1) as wpool,
            tc.tile_pool(name="acts", bufs=1) as apool,
            tc.tile_pool(name="qkv", bufs=1) as qkvpool,
            tc.tile_pool(name="vsb", bufs=MT) as vpool,
            tc.tile_pool(name="attn", bufs=6) as attnpool,
            tc.tile_pool(name="small", bufs=4) as rpool,
            tc.tile_pool(name="stage", bufs=3) as stpool,
        ):
            # ---- load weights + activations -------------------------------
            # DMA issue order = priority order: the Q-projection needs only
            # wq + pixT, so those go first and compute starts ~5us in while
            # the K/V/O weights and patT stream behind them.
            wq_sb = wpool.tile([128, CC, INNER], BF16, tag="wq")
            wk_sb = wpool.tile([128, CC, INNER], BF16, tag="wk")
            wv_sb = wpool.tile([128, CC, INNER], BF16, tag="wv")
            wo_sb = wpool.tile([128, IC, CDIM], BF16, tag="wo")
            bo_sb = wpool.tile([1, CDIM], BF16, tag="bo")

            pixT = apool.tile([128, CC, NSH], BF16, tag="pixT")
            patT = apool.tile([128, CC, M], BF16, tag="patT")
            pix_r = pixelT.rearrange("(cc p) n -> p cc n", p=128)
            pat_r = patchT.rearrange("(cc p) m -> p cc m", p=128)

            nc.sync.dma_start(
                bo_sb,
                bass.AP(tensor=bo[:].tensor, offset=0, ap=[[0, 1], [1, CDIM]]),
            )
            wq_r = wq.rearrange("(cc p) i -> p cc i", p=128)
            for cc in range(CC):
                nc.sync.dma_start(pixT[:, cc, 0:512], pix_r[:, cc, 0:512])
                nc.sync.dma_start(wq_sb[:, cc, :], wq_r[:, cc, :])
            for cc in range(CC):
                nc.sync.dma_start(pixT[:, cc, 512:1024], pix_r[:, cc, 512:1024])
            wk_r = wk.rearrange("(cc p) i -> p cc i", p=128)
            for cc in range(CC):
                nc.sync.dma_start(patT[:, cc, 0:1024], pat_r[:, cc, 0:1024])
                nc.sync.dma_start(patT[:, cc, 1024:2048], pat_r[:, cc, 1024:2048])
                nc.gpsimd.dma_start(wk_sb[:, cc, :], wk_r[:, cc, :])
            nc.gpsimd.dma_start(wv_sb, wv.rearrange("(cc p) i -> p cc i", p=128))
            nc.gpsimd.dma_start(wo_sb, wo.rearrange("(ic p) o -> p ic o", p=128))

            # warm the exp table early so the first real exp isn't gated on it
            warm = rpool.tile([1, 16], BF16, tag="warm")
            nc.scalar.activation(
                warm, bo_sb[0:1, 0:16], mybir.ActivationFunctionType.Exp
            )
            # ones row: adds the bias via a k=1 matmul in the output projection
            ones_sb = wpool.tile([1, 128], BF16, tag="ones")
            nc.gpsimd.memset(ones_sb, 1.0)

            qT = qkvpool.tile([128, IC, NSH], BF16, tag="qT")
            # kTp: per head a full-k=128 stationary — the head's K^T in its own
            # 64-row range, zeros in the other head's rows. Streaming cost of a
            # matmul is N cycles regardless of k, and the full-height stationary
            # keeps the PE activity monitor (HAM) at the 2.4 GHz clock.
            kTp = qkvpool.tile([128, IC, 2, M], BF16, tag="kTp")
            for ic in range(IC):
                nc.gpsimd.memset(kTp[D : 2 * D, ic, 0, :], 0.0)
                nc.gpsimd.memset(kTp[0:D, ic, 1, :], 0.0)
            # v_sb: [m-chunk 128, head, 128] = [V_h | 1 | zeros] — col 64 gives
            # the softmax denominator via the matmul, cols 65..127 pad M to 128.
            v_sb = [
                vpool.tile([128, H, 128], BF16, tag="v", name=f"v{mi}")
                for mi in range(MT)
            ]
            for mi in range(MT):
                nc.gpsimd.memset(v_sb[mi][:, :, D : 2 * D], 0.0)
                nc.gpsimd.memset(v_sb[mi][:, :, D : D + 1], 1.0)

            # ---- projections ---------------------------------------------
            with tc.tile_pool(name="ppsum", bufs=3, space="PSUM") as ppsum:
                # Q^T [inner, n]; PSUM->SBUF drains alternate DVE/ScalarE so
                # neither engine is the projection-phase bottleneck.
                for ic in range(IC):
                    for nj in range(NJ):
                        ps = ppsum.tile([128, 512], F32, tag="p")
                        for cc in range(CC):
                            nc.tensor.matmul(
                                ps,
                                wq_sb[:, cc, ic * 128 : (ic + 1) * 128],
                                pixT[:, cc, nj * 512 : (nj + 1) * 512],
                                start=(cc == 0),
                                stop=(cc == CC - 1),
                            )
                        dst = qT[:, ic, nj * 512 : (nj + 1) * 512]
                        if (ic * NJ + nj) % 2 == 0:
                            nc.vector.tensor_copy(dst, ps)
                        else:
                            nc.scalar.copy(dst, ps)
                # K^T [inner, m] -> zero-padded per-head stationaries
                for ic in range(IC):
                    for mj in range(MJ):
                        ps = ppsum.tile([128, 512], F32, tag="p")
                        for cc in range(CC):
                            nc.tensor.matmul(
                                ps,
                                wk_sb[:, cc, ic * 128 : (ic + 1) * 128],
                                patT[:, cc, mj * 512 : (mj + 1) * 512],
                                start=(cc == 0),
                                stop=(cc == CC - 1),
                            )
                        sl = slice(mj * 512, (mj + 1) * 512)
                        nc.vector.tensor_copy(kTp[0:D, ic, 0, sl], ps[0:D, :])
                        nc.scalar.copy(kTp[D : 2 * D, ic, 1, sl], ps[D : 2 * D, :])
                # V natural [m, inner] -> per-m-chunk [128, H, D+1] with ones col
                for mi in range(MT):
                    ps = ppsum.tile([128, 512], F32, tag="p")
                    for cc in range(CC):
                        nc.tensor.matmul(
                            ps,
                            patT[:, cc, mi * 128 : (mi + 1) * 128],
                            wv_sb[:, cc, :],
                            start=(cc == 0),
                            stop=(cc == CC - 1),
                        )
                    dst = v_sb[mi][:, :, 0:D]
                    src = ps.rearrange("p (h d) -> p h d", h=H)
                    if mi % 2 == 0:
                        nc.vector.tensor_copy(dst, src)
                    else:
                        nc.scalar.copy(dst, src)

            # ---- attention (flat software-pipelined loop) -----------------
            # Iterations t = h*MT + mi; the attn-out matmul for step t runs
            # one iteration behind the scores matmul, so the tensor engine
            # never stalls in-order behind an exp it doesn't yet need.
            outT = qkvpool.tile([128, IC, NSH], BF16, tag="outT")

            def normalize(h, o_ps):
                """o_ps rows 0:D hold unnormalized attn-out, row D the softmax
                denominator. 1/denom = exp(-ln(denom)) on ScalarE (both live in
                the natural_log_exp table set, so no table thrash with the
                attention exps); broadcast across the D output partitions on
                GpSimd; multiply on DVE."""
                ic = h // 2
                po = (h % 2) * D
                for nj in range(NJ):
                    u = rpool.tile([1, 512], F32, tag="u")
                    nc.scalar.activation(
                        u, o_ps[nj][D : D + 1, :], mybir.ActivationFunctionType.Ln
                    )
                    r = rpool.tile([1, 512], F32, tag="r")
                    nc.scalar.activation(
                        r, u, mybir.ActivationFunctionType.Exp, scale=-1.0
                    )
                    r64 = rpool.tile([D, 512], F32, tag="r64")
                    nc.gpsimd.partition_broadcast(r64, r[0:1, :], channels=D)
                    nc.vector.tensor_mul(
                        outT[po : po + D, ic, nj * 512 : (nj + 1) * 512],
                        o_ps[nj][0:D, :],
                        r64,
                    )

            with (
                tc.tile_pool(name="spsum", bufs=2, space="PSUM") as spsum,
                tc.tile_pool(name="opsum", bufs=4, space="PSUM") as opsum,
            ):
                o_ps_all = {}
                pending = []  # [(h, mi, at)] attn-out matmuls lagging 2 iters
                LAG = 2

                def drain_one():
                    ph, pmi, pat = pending.pop(0)
                    for nj in range(NJ):
                        nc.tensor.matmul(
                            o_ps_all[ph][nj],
                            v_sb[pmi][:, ph, :],
                            pat[:, nj * 512 : (nj + 1) * 512],
                            start=(pmi == 0),
                            stop=(pmi == MT - 1),
                        )
                    if pmi == MT - 1:
                        normalize(ph, o_ps_all.pop(ph))

                for t in range(H * MT):
                    h, mi = divmod(t, MT)
                    ic = h // 2
                    if mi == 0:
                        o_ps_all[h] = [
                            opsum.tile([128, 512], F32, tag="o", name=f"o{h}_{nj}")
                            for nj in range(NJ)
                        ]
                    s_ps = spsum.tile([128, NJ * 512], F32, tag="s")
                    for nj in range(NJ):
                        nc.tensor.matmul(
                            s_ps[:, nj * 512 : (nj + 1) * 512],
                            kTp[:, ic, h % 2, mi * 128 : (mi + 1) * 128],
                            qT[:, ic, nj * 512 : (nj + 1) * 512],
                            start=True,
                            stop=True,
                        )
                    at = attnpool.tile([128, NJ * 512], BF16, tag="at")
                    if mi % 2 == 0:
                        nc.scalar.activation(
                            at, s_ps, mybir.ActivationFunctionType.Exp, scale=SCALE
                        )
                    else:
                        # split the exp across ScalarE (exact) and DVE
                        # (Schraudolph bit-trick) so neither engine gates
                        # the attention pipeline
                        nc.scalar.activation(
                            at[:, 0:512],
                            s_ps[:, 0:512],
                            mybir.ActivationFunctionType.Exp,
                            scale=SCALE,
                        )
                        nc.vector.tensor_scalar(
                            at[:, 512:1024].bitcast(mybir.dt.int16),
                            s_ps[:, 512:1024],
                            EXPA,
                            EXPB,
                            mybir.AluOpType.mult,
                            mybir.AluOpType.add,
                        )
                    pending.append((h, mi, at))
                    if len(pending) > LAG:
                        drain_one()
                while pending:
                    drain_one()

            # ---- output projection ---------------------------------------
            # bias is added by a k=1 matmul (ones stationary, bo moving) so the
            # PSUM->SBUF drain is a plain ScalarE copy off the critical DVE path.
            with tc.tile_pool(name="fpsum", bufs=2, space="PSUM") as fpsum:
                for ni in range(NT):
                    ps = fpsum.tile([128, CDIM], F32, tag="f")
                    nc.tensor.matmul(
                        ps,
                        ones_sb[0:1, 0:128],
                        bo_sb,
                        start=True,
                        stop=False,
                    )
                    for ic in range(IC):
                        nc.tensor.matmul(
                            ps,
                            outT[:, ic, ni * 128 : (ni + 1) * 128],
                            wo_sb[:, ic, :],
                            start=False,
                            stop=(ic == IC - 1),
                        )
                    st = stpool.tile([128, CDIM], F32, tag="st")
                    if ni % 2 == 0:
                        nc.scalar.copy(st, ps)
                    else:
                        nc.vector.tensor_copy(st, ps)
                    nc.sync.dma_start(out[ni * 128 : (ni + 1) * 128, :], st)

    nc.finalize()
    return nc


def make_in_maps(pixel_embed, patch_embed, Wq, Wk, Wv, Wo, bo):
    bf = ml_dtypes.bfloat16
    pixel_embed = np.asarray(pixel_embed, dtype=np.float32)
    patch_embed = np.asarray(patch_embed, dtype=np.float32)
    wq = np.asarray(Wq, dtype=np.float32).astype(bf)
    wk = np.asarray(Wk, dtype=np.float32).astype(bf)
    wv = np.asarray(Wv, dtype=np.float32).astype(bf)
    wo = np.asarray(Wo, dtype=np.float32).astype(bf)
    bo = np.asarray(bo, dtype=np.float32).astype(bf)

    in_maps = []
    for core in range(N_CORES):
        bi, half = divmod(core, 2)
        px = pixel_embed[bi, half * NSH : (half + 1) * NSH, :]  # [NSH, CDIM]
        pa = patch_embed[bi]  # [M, CDIM]
        in_maps.append(
            {
                "pixelT": px.T.astype(bf),
                "patchT": pa.T.astype(bf),
                "wq": wq,
                "wk": wk,
                "wv": wv,
                "wo": wo,
                "bo": bo,
            }
        )
    return in_maps


def gather_out(results):
    out = np.empty((B, N, CDIM), np.float32)
    for core in range(N_CORES):
        bi, half = divmod(core, 2)
        out[bi, half * NSH : (half + 1) * NSH, :] = results[core]["out"]
    return out


_NC_CACHE = {}


def kernel(pixel_embed, patch_embed, Wq, Wk, Wv, Wo, bo, **kw):
    if "nc" not in _NC_CACHE:
        _NC_CACHE["nc"] = build_nc()
    nc = _NC_CACHE["nc"]
    in_maps = make_in_maps(pixel_embed, patch_embed, Wq, Wk, Wv, Wo, bo)
    res = run_bass_kernel_spmd(nc, in_maps, core_ids=list(range(N_CORES)), **kw)
    out = gather_out(res.results)
    if kw.get("trace"):
        return out, res
    return out



# revision 18
# speedup vs baseline: 1.4644x; 1.4644x over previous
"""Cross-attention kernel for Trainium2, sharded over 8 NeuronCores.

Problem (hardcoded): b=4, n=m=2048, query_dim=context_dim=512,
heads=8, dim_head=64 (inner=512), f32 I/O.

Sharding: data-parallel over (batch, query-half): core c -> batch c//2,
query rows [(c%2)*1024, (c%2+1)*1024). Each core holds the full K/V
context for its batch, so there are no collectives and output shards
tile the full output exactly.

Layout strategy (all matmul inputs bf16, accumulation f32 in PSUM):
  - Host pre-transposes activations: pixelT [512c, 1024n], patchT [512c, 2048m].
  - Q^T [inner, n] and K^T [inner, m] computed with weights as stationary.
  - V kept natural [m, inner], stored per m-chunk as [128, 8 heads, 65]
    with a constant-1 column appended per head: the attention-output
    matmul then yields [65, n] per head where row 64 = sum(exp(scores)),
    giving the softmax denominator for free.
  - scores^T [m, n] per head via k=64 matmuls (kT stationary, qT moving);
    exp runs on ScalarE directly PSUM->SBUF(bf16) with scale=1/8 folded in.
  - attn-out^T [65, n] accumulates over 16 m-chunks in PSUM; normalized by
    reciprocal of row 64 (broadcast across partitions via gpsimd).
  - Final projection: outT (inner on partitions) stationary, Wo moving;
    bias added on the PSUM->SBUF copy.
"""

import numpy as np
import ml_dtypes

import concourse.bass as bass
import concourse.mybir as mybir
import concourse.tile as tile
from concourse import bacc
from concourse.bass_utils import run_bass_kernel_spmd

BF16 = mybir.dt.bfloat16
F32 = mybir.dt.float32

B, N, M = 4, 2048, 2048
CDIM, INNER = 512, 512
H, D = 8, 64
NSH = N // 2  # query rows per core
N_CORES = 8
SCALE = D ** -0.5

CC = CDIM // 128   # contraction chunks for projections (4)
IC = INNER // 128  # inner-dim chunks (4)
MT = M // 128      # m tiles (16)
NJ = NSH // 512    # n chunks of 512 (2)
NT = NSH // 128    # n tiles (8)
MJ = M // 512      # m chunks of 512 (4)

# Schraudolph exp-as-bit-trick constants for the DVE share of the softmax:
# bf16 bits of exp(s*SCALE) ~= s*EXPA + EXPB (computed f32, converted to i16,
# bitcast to bf16). Max rel err ~3.3% (floor) / 3.5% (round-to-nearest) on the
# attention weights, which washes out in the softmax-weighted average.
EXPA = 16 * float(np.log2(np.e))
EXPB = 16250.9


def build_nc() -> bass.Bass:
    nc = bacc.Bacc(None)

    pixelT = nc.dram_tensor("pixelT", [CDIM, NSH], BF16, kind="ExternalInput")
    patchT = nc.dram_tensor("patchT", [CDIM, M], BF16, kind="ExternalInput")
    wq = nc.dram_tensor("wq", [CDIM, INNER], BF16, kind="ExternalInput")
    wk = nc.dram_tensor("wk", [CDIM, INNER], BF16, kind="ExternalInput")
    wv = nc.dram_tensor("wv", [CDIM, INNER], BF16, kind="ExternalInput")
    wo = nc.dram_tensor("wo", [INNER, CDIM], BF16, kind="ExternalInput")
    bo = nc.dram_tensor("bo", [CDIM], BF16, kind="ExternalInput")
    out = nc.dram_tensor("out", [NSH, CDIM], F32, kind="ExternalOutput")

    with tile.TileContext(nc) as tc:
        with (
            tc.tile_pool(name="weights", bufs=1) as wpool,
            tc.tile_pool(name="acts", bufs=1) as apool,
            tc.tile_pool(name="qkv", bufs=1) as qkvpool,
            tc.tile_pool(name="vsb", bufs=MT) as vpool,
            tc.tile_pool(name="attn", bufs=6) as attnpool,
            tc.tile_pool(name="small", bufs=4) as rpool,
            tc.tile_pool(name="stage", bufs=3) as stpool,
        ):
            # ---- load weights + activations -------------------------------
            # DMA issue order = priority order: the Q-projection needs only
            # wq + pixT, so those go first and compute starts ~5us in while
            # the K/V/O weights and patT stream behind them.
            wq_sb = wpool.tile([128, CC, INNER], BF16, tag="wq")
            wk_sb = wpool.tile([128, CC, INNER], BF16, tag="wk")
            wv_sb = wpool.tile([128, CC, INNER], BF16, tag="wv")
            wo_sb = wpool.tile([128, IC, CDIM], BF16, tag="wo")
            bo_sb = wpool.tile([1, CDIM], BF16, tag="bo")

            pixT = apool.tile([128, CC, NSH], BF16, tag="pixT")
            patT = apool.tile([128, CC, M], BF16, tag="patT")
            pix_r = pixelT.rearrange("(cc p) n -> p cc n", p=128)
            pat_r = patchT.rearrange("(cc p) m -> p cc m", p=128)

            nc.sync.dma_start(
                bo_sb,
                bass.AP(tensor=bo[:].tensor, offset=0, ap=[[0, 1], [1, CDIM]]),
            )
            wq_r = wq.rearrange("(cc p) i -> p cc i", p=128)
            for cc in range(CC):
                nc.sync.dma_start(pixT[:, cc, 0:512], pix_r[:, cc, 0:512])
                nc.sync.dma_start(wq_sb[:, cc, :], wq_r[:, cc, :])
            for cc in range(CC):
                nc.sync.dma_start(pixT[:, cc, 512:1024], pix_r[:, cc, 512:1024])
            wk_r = wk.rearrange("(cc p) i -> p cc i", p=128)
            for cc in range(CC):
                nc.sync.dma_start(patT[:, cc, 0:1024], pat_r[:, cc, 0:1024])
                nc.sync.dma_start(patT[:, cc, 1024:2048], pat_r[:, cc, 1024:2048])
                nc.gpsimd.dma_start(wk_sb[:, cc, :], wk_r[:, cc, :])
            nc.gpsimd.dma_start(wv_sb, wv.rearrange("(cc p) i -> p cc i", p=128))
            nc.gpsimd.dma_start(wo_sb, wo.rearrange("(ic p) o -> p ic o", p=128))

            # warm the exp table early so the first real exp isn't gated on it
            warm = rpool.tile([1, 16], BF16, tag="warm")
            nc.scalar.activation(
                warm, bo_sb[0:1, 0:16], mybir.ActivationFunctionType.Exp
            )
            # ones row: adds the bias via a k=1 matmul in the output projection
            ones_sb = wpool.tile([1, 128], BF16, tag="ones")
            nc.gpsimd.memset(ones_sb, 1.0)

            qT = qkvpool.tile([128, IC, NSH], BF16, tag="qT")
            # kTp: per head a full-k=128 stationary — the head's K^T in its own
            # 64-row range, zeros in the other head's rows. Streaming cost of a
            # matmul is N cycles regardless of k, and the full-height stationary
            # keeps the PE activity monitor (HAM) at the 2.4 GHz clock.
            kTp = qkvpool.tile([128, IC, 2, M], BF16, tag="kTp")
            for ic in range(IC):
                nc.gpsimd.memset(kTp[D : 2 * D, ic, 0, :], 0.0)
                nc.gpsimd.memset(kTp[0:D, ic, 1, :], 0.0)
            # v_sb: [m-chunk 128, head, 128] = [V_h | 1 | zeros] — col 64 gives
            # the softmax denominator via the matmul, cols 65..127 pad M to 128.
            v_sb = [
                vpool.tile([128, H, 128], BF16, tag="v", name=f"v{mi}")
                for mi in range(MT)
            ]
            for mi in range(MT):
                nc.gpsimd.memset(v_sb[mi][:, :, D : 2 * D], 0.0)
                nc.gpsimd.memset(v_sb[mi][:, :, D : D + 1], 1.0)

            # ---- projections ---------------------------------------------
            with tc.tile_pool(name="ppsum", bufs=3, space="PSUM") as ppsum:
                # Q^T [inner, n]; PSUM->SBUF drains alternate DVE/ScalarE so
                # neither engine is the projection-phase bottleneck.
                for ic in range(IC):
                    for nj in range(NJ):
                        ps = ppsum.tile([128, 512], F32, tag="p")
                        for cc in range(CC):
                            nc.tensor.matmul(
                                ps,
                                wq_sb[:, cc, ic * 128 : (ic + 1) * 128],
                                pixT[:, cc, nj * 512 : (nj + 1) * 512],
                                start=(cc == 0),
                                stop=(cc == CC - 1),
                            )
                        dst = qT[:, ic, nj * 512 : (nj + 1) * 512]
                        if (ic * NJ + nj) % 2 == 0:
                            nc.vector.tensor_copy(dst, ps)
                        else:
                            nc.scalar.copy(dst, ps)
                # K^T [inner, m] -> zero-padded per-head stationaries
                for ic in range(IC):
                    for mj in range(MJ):
                        ps = ppsum.tile([128, 512], F32, tag="p")
                        for cc in range(CC):
                            nc.tensor.matmul(
                                ps,
                                wk_sb[:, cc, ic * 128 : (ic + 1) * 128],
                                patT[:, cc, mj * 512 : (mj + 1) * 512],
                                start=(cc == 0),
                                stop=(cc == CC - 1),
                            )
                        sl = slice(mj * 512, (mj + 1) * 512)
                        nc.vector.tensor_copy(kTp[0:D, ic, 0, sl], ps[0:D, :])
                        nc.scalar.copy(kTp[D : 2 * D, ic, 1, sl], ps[D : 2 * D, :])
                # V natural [m, inner] -> per-m-chunk [128, H, D+1] with ones col
                for mi in range(MT):
                    ps = ppsum.tile([128, 512], F32, tag="p")
                    for cc in range(CC):
                        nc.tensor.matmul(
                            ps,
                            patT[:, cc, mi * 128 : (mi + 1) * 128],
                            wv_sb[:, cc, :],
                            start=(cc == 0),
                            stop=(cc == CC - 1),
                        )
                    dst = v_sb[mi][:, :, 0:D]
                    src = ps.rearrange("p (h d) -> p h d", h=H)
                    if mi % 2 == 0:
                        nc.vector.tensor_copy(dst, src)
                    else:
                        nc.scalar.copy(dst, src)

            # ---- attention (flat software-pipelined loop) -----------------
            # Iterations t = h*MT + mi; the attn-out matmul for step t runs
            # one iteration behind the scores matmul, so the tensor engine
            # never stalls in-order behind an exp it doesn't yet need.
            outT = qkvpool.tile([128, IC, NSH], BF16, tag="outT")

            def normalize(h, o_ps):
                """o_ps rows 0:D hold unnormalized attn-out, row D the softmax
                denominator, rows D+1.. zeros. Transpose 32x32 blocks so the
                denominator row spreads across 32 partitions (reciprocal then
                costs FD=16, not FD=512), transpose back, broadcast across the
                D output partitions on GpSimd, multiply on DVE. Both nj chains
                are interleaved so the GpSimd broadcasts overlap DVE work."""
                ic = h // 2
                po = (h % 2) * D
                r2s = []
                for nj in range(NJ):
                    rt = rpool.tile([32, 16, 32], F32, tag="rt")
                    rt_flat = rt.rearrange("p a b -> p (a b)")
                    nc.vector.transpose(rt_flat, o_ps[nj][D : D + 32, :])
                    nc.vector.reciprocal(rt[:, :, 0:1], rt[:, :, 0:1])
                    r2 = rpool.tile([32, 512], F32, tag="r2")
                    nc.vector.transpose(r2, rt_flat)
                    r2s.append(r2)
                r64s = []
                for nj in range(NJ):
                    r64 = rpool.tile([D, 512], F32, tag="r64")
                    nc.gpsimd.partition_broadcast(r64, r2s[nj][0:1, :], channels=D)
                    r64s.append(r64)
                for nj in range(NJ):
                    nc.vector.tensor_mul(
                        outT[po : po + D, ic, nj * 512 : (nj + 1) * 512],
                        o_ps[nj][0:D, :],
                        r64s[nj],
                    )

            with (
                tc.tile_pool(name="spsum", bufs=2, space="PSUM") as spsum,
                tc.tile_pool(name="opsum", bufs=4, space="PSUM") as opsum,
            ):
                o_ps_all = {}
                pending = []  # [(h, mi, at)] attn-out matmuls lagging 2 iters
                LAG = 2

                def drain_one():
                    ph, pmi, pat = pending.pop(0)
                    for nj in range(NJ):
                        nc.tensor.matmul(
                            o_ps_all[ph][nj],
                            v_sb[pmi][:, ph, :],
                            pat[:, nj * 512 : (nj + 1) * 512],
                            start=(pmi == 0),
                            stop=(pmi == MT - 1),
                        )
                    if pmi == MT - 1:
                        normalize(ph, o_ps_all.pop(ph))

                for t in range(H * MT):
                    h, mi = divmod(t, MT)
                    ic = h // 2
                    if mi == 0:
                        o_ps_all[h] = [
                            opsum.tile([128, 512], F32, tag="o", name=f"o{h}_{nj}")
                            for nj in range(NJ)
                        ]
                    s_ps = spsum.tile([128, NJ * 512], F32, tag="s")
                    for nj in range(NJ):
                        nc.tensor.matmul(
                            s_ps[:, nj * 512 : (nj + 1) * 512],
                            kTp[:, ic, h % 2, mi * 128 : (mi + 1) * 128],
                            qT[:, ic, nj * 512 : (nj + 1) * 512],
                            start=True,
                            stop=True,
                        )
                    at = attnpool.tile([128, NJ * 512], BF16, tag="at")
                    if mi % 2 == 0:
                        nc.scalar.activation(
                            at, s_ps, mybir.ActivationFunctionType.Exp, scale=SCALE
                        )
                    else:
                        # split the exp across ScalarE (exact) and DVE
                        # (Schraudolph bit-trick) so neither engine gates
                        # the attention pipeline
                        nc.scalar.activation(
                            at[:, 0:512],
                            s_ps[:, 0:512],
                            mybir.ActivationFunctionType.Exp,
                            scale=SCALE,
                        )
                        nc.vector.tensor_scalar(
                            at[:, 512:1024].bitcast(mybir.dt.int16),
                            s_ps[:, 512:1024],
                            EXPA,
                            EXPB,
                            mybir.AluOpType.mult,
                            mybir.AluOpType.add,
                        )
                    pending.append((h, mi, at))
                    if len(pending) > LAG:
                        drain_one()
                while pending:
                    drain_one()

            # ---- output projection ---------------------------------------
            # bias is added by a k=1 matmul (ones stationary, bo moving) so the
            # PSUM->SBUF drain is a plain ScalarE copy off the critical DVE path.
            with tc.tile_pool(name="fpsum", bufs=2, space="PSUM") as fpsum:
                for ni in range(NT):
                    ps = fpsum.tile([128, CDIM], F32, tag="f")
                    nc.tensor.matmul(
                        ps,
                        ones_sb[0:1, 0:128],
                        bo_sb,
                        start=True,
                        stop=False,
                    )
                    for ic in range(IC):
                        nc.tensor.matmul(
                            ps,
                            outT[:, ic, ni * 128 : (ni + 1) * 128],
                            wo_sb[:, ic, :],
                            start=False,
                            stop=(ic == IC - 1),
                        )
                    st = stpool.tile([128, CDIM], F32, tag="st")
                    if ni % 2 == 0:
                        nc.scalar.copy(st, ps)
                    else:
                        nc.vector.tensor_copy(st, ps)
                    nc.sync.dma_start(out[ni * 128 : (ni + 1) * 128, :], st)

    nc.finalize()
    return nc


def make_in_maps(pixel_embed, patch_embed, Wq, Wk, Wv, Wo, bo):
    bf = ml_dtypes.bfloat16
    pixel_embed = np.asarray(pixel_embed, dtype=np.float32)
    patch_embed = np.asarray(patch_embed, dtype=np.float32)
    wq = np.asarray(Wq, dtype=np.float32).astype(bf)
    wk = np.asarray(Wk, dtype=np.float32).astype(bf)
    wv = np.asarray(Wv, dtype=np.float32).astype(bf)
    wo = np.asarray(Wo, dtype=np.float32).astype(bf)
    bo = np.asarray(bo, dtype=np.float32).astype(bf)

    in_maps = []
    for core in range(N_CORES):
        bi, half = divmod(core, 2)
        px = pixel_embed[bi, half * NSH : (half + 1) * NSH, :]  # [NSH, CDIM]
        pa = patch_embed[bi]  # [M, CDIM]
        in_maps.append(
            {
                "pixelT": px.T.astype(bf),
                "patchT": pa.T.astype(bf),
                "wq": wq,
                "wk": wk,
                "wv": wv,
                "wo": wo,
                "bo": bo,
            }
        )
    return in_maps


def gather_out(results):
    out = np.empty((B, N, CDIM), np.float32)
    for core in range(N_CORES):
        bi, half = divmod(core, 2)
        out[bi, half * NSH : (half + 1) * NSH, :] = results[core]["out"]
    return out


_NC_CACHE = {}


def kernel(pixel_embed, patch_embed, Wq, Wk, Wv, Wo, bo, **kw):
    if "nc" not in _NC_CACHE:
        _NC_CACHE["nc"] = build_nc()
    nc = _NC_CACHE["nc"]
    in_maps = make_in_maps(pixel_embed, patch_embed, Wq, Wk, Wv, Wo, bo)
    res = run_bass_kernel_spmd(nc, in_maps, core_ids=list(range(N_CORES)), **kw)
    out = gather_out(res.results)
    if kw.get("trace"):
        return out, res
    return out



# revision 23
# speedup vs baseline: 1.5068x; 1.0289x over previous
"""Cross-attention kernel for Trainium2, sharded over 8 NeuronCores.

Problem (hardcoded): b=4, n=m=2048, query_dim=context_dim=512,
heads=8, dim_head=64 (inner=512), f32 I/O.

Sharding: data-parallel over (batch, query-half): core c -> batch c//2,
query rows [(c%2)*1024, (c%2+1)*1024). Each core holds the full K/V
context for its batch, so there are no collectives and output shards
tile the full output exactly.

Layout strategy (all matmul inputs bf16, accumulation f32 in PSUM):
  - Host pre-transposes activations: pixelT [512c, 1024n], patchT [512c, 2048m].
  - Q^T [inner, n] and K^T [inner, m] computed with weights as stationary.
  - V kept natural [m, inner], stored per m-chunk as [128, 8 heads, 65]
    with a constant-1 column appended per head: the attention-output
    matmul then yields [65, n] per head where row 64 = sum(exp(scores)),
    giving the softmax denominator for free.
  - scores^T [m, n] per head via k=64 matmuls (kT stationary, qT moving);
    exp runs on ScalarE directly PSUM->SBUF(bf16) with scale=1/8 folded in.
  - attn-out^T [65, n] accumulates over 16 m-chunks in PSUM; normalized by
    reciprocal of row 64 (broadcast across partitions via gpsimd).
  - Final projection: outT (inner on partitions) stationary, Wo moving;
    bias added on the PSUM->SBUF copy.
"""

import numpy as np
import ml_dtypes

import concourse.bass as bass
import concourse.mybir as mybir
import concourse.tile as tile
from concourse import bacc
from concourse.bass_utils import run_bass_kernel_spmd

BF16 = mybir.dt.bfloat16
F32 = mybir.dt.float32

B, N, M = 4, 2048, 2048
CDIM, INNER = 512, 512
H, D = 8, 64
NSH = N // 2  # query rows per core
N_CORES = 8
SCALE = D ** -0.5

CC = CDIM // 128   # contraction chunks for projections (4)
IC = INNER // 128  # inner-dim chunks (4)
MT = M // 128      # m tiles (16)
NJ = NSH // 512    # n chunks of 512 (2)
NT = NSH // 128    # n tiles (8)
MJ = M // 512      # m chunks of 512 (4)

# Schraudolph exp-as-bit-trick constants for the DVE share of the softmax:
# bf16 bits of exp(s*SCALE) ~= s*EXPA + EXPB (computed f32, converted to i16,
# bitcast to bf16). Max rel err ~3.3% (floor) / 3.5% (round-to-nearest) on the
# attention weights, which washes out in the softmax-weighted average.
EXPA = 16 * float(np.log2(np.e))
EXPB = 16250.9


def build_nc() -> bass.Bass:
    nc = bacc.Bacc(None)

    pixelT = nc.dram_tensor("pixelT", [CDIM, NSH], BF16, kind="ExternalInput")
    patchT = nc.dram_tensor("patchT", [CDIM, M], BF16, kind="ExternalInput")
    wq = nc.dram_tensor("wq", [CDIM, INNER], BF16, kind="ExternalInput")
    wk = nc.dram_tensor("wk", [CDIM, INNER], BF16, kind="ExternalInput")
    wv = nc.dram_tensor("wv", [CDIM, INNER], BF16, kind="ExternalInput")
    wo = nc.dram_tensor("wo", [INNER, CDIM], BF16, kind="ExternalInput")
    bo = nc.dram_tensor("bo", [CDIM], BF16, kind="ExternalInput")
    out = nc.dram_tensor("out", [NSH, CDIM], F32, kind="ExternalOutput")

    with tile.TileContext(nc) as tc:
        with (
            tc.tile_pool(name="weights", bufs=1) as wpool,
            tc.tile_pool(name="acts", bufs=1) as apool,
            tc.tile_pool(name="qkv", bufs=1) as qkvpool,
            tc.tile_pool(name="vsb", bufs=MT) as vpool,
            tc.tile_pool(name="attn", bufs=6) as attnpool,
            tc.tile_pool(name="small", bufs=4) as rpool,
            tc.tile_pool(name="stage", bufs=3) as stpool,
        ):
            # ---- load weights + activations -------------------------------
            # DMA issue order = priority order: the Q-projection needs only
            # wq + pixT, so those go first and compute starts ~5us in while
            # the K/V/O weights and patT stream behind them.
            wq_sb = wpool.tile([128, CC, INNER], BF16, tag="wq")
            wk_sb = wpool.tile([128, CC, INNER], BF16, tag="wk")
            wv_sb = wpool.tile([128, CC, INNER], BF16, tag="wv")
            wo_sb = wpool.tile([128, IC, CDIM], BF16, tag="wo")
            bo_sb = wpool.tile([1, CDIM], BF16, tag="bo")

            pixT = apool.tile([128, CC, NSH], BF16, tag="pixT")
            patT = apool.tile([128, CC, M], BF16, tag="patT")
            pix_r = pixelT.rearrange("(cc p) n -> p cc n", p=128)
            pat_r = patchT.rearrange("(cc p) m -> p cc m", p=128)

            nc.sync.dma_start(
                bo_sb,
                bass.AP(tensor=bo[:].tensor, offset=0, ap=[[0, 1], [1, CDIM]]),
            )
            # K-projection inputs first (it runs first), in mj-sized pieces so
            # the first matmuls start after ~1MB instead of ~5MB of DMA.
            wk_r = wk.rearrange("(cc p) i -> p cc i", p=128)
            for cc in range(CC):
                nc.sync.dma_start(wk_sb[:, cc, :], wk_r[:, cc, :])
            for mj in range(MJ):
                for cc in range(CC):
                    sl = slice(mj * 512, (mj + 1) * 512)
                    nc.sync.dma_start(patT[:, cc, sl], pat_r[:, cc, sl])
            wq_r = wq.rearrange("(cc p) i -> p cc i", p=128)
            for cc in range(CC):
                nc.sync.dma_start(wq_sb[:, cc, :], wq_r[:, cc, :])
            for nj in range(NJ):
                for cc in range(CC):
                    sl = slice(nj * 512, (nj + 1) * 512)
                    nc.sync.dma_start(pixT[:, cc, sl], pix_r[:, cc, sl])
            nc.gpsimd.dma_start(wv_sb, wv.rearrange("(cc p) i -> p cc i", p=128))
            nc.gpsimd.dma_start(wo_sb, wo.rearrange("(ic p) o -> p ic o", p=128))

            # warm the exp table early so the first real exp isn't gated on it
            warm = rpool.tile([1, 16], BF16, tag="warm")
            nc.scalar.activation(
                warm, bo_sb[0:1, 0:16], mybir.ActivationFunctionType.Exp
            )
            # ones row: adds the bias via a k=1 matmul in the output projection
            ones_sb = wpool.tile([1, 128], BF16, tag="ones")
            nc.gpsimd.memset(ones_sb, 1.0)

            qT = qkvpool.tile([128, IC, NSH], BF16, tag="qT")
            # kTp: per head a full-k=128 stationary — the head's K^T in its own
            # 64-row range, zeros in the other head's rows. Streaming cost of a
            # matmul is N cycles regardless of k, and the full-height stationary
            # keeps the PE activity monitor (HAM) at the 2.4 GHz clock.
            kTp = qkvpool.tile([128, IC, 2, M], BF16, tag="kTp")
            for ic in range(IC):
                nc.gpsimd.memset(kTp[D : 2 * D, ic, 0, :], 0.0)
                nc.gpsimd.memset(kTp[0:D, ic, 1, :], 0.0)
            # v_sb: [m-chunk 128, head, 128] = [V_h | 1 | zeros] — col 64 gives
            # the softmax denominator via the matmul, cols 65..127 pad M to 128.
            v_sb = [
                vpool.tile([128, H, 128], BF16, tag="v", name=f"v{mi}")
                for mi in range(MT)
            ]
            for mi in range(MT):
                nc.gpsimd.memset(v_sb[mi][:, :, D : 2 * D], 0.0)
                nc.gpsimd.memset(v_sb[mi][:, :, D : D + 1], 1.0)

            # ---- projections ---------------------------------------------
            with tc.tile_pool(name="ppsum", bufs=3, space="PSUM") as ppsum:
                # K^T [inner, m] -> zero-padded per-head stationaries. Runs
                # first (its DMAs land first). PSUM->SBUF drains split between
                # DVE and ScalarE so neither gates the projection phase.
                for ic in range(IC):
                    for mj in range(MJ):
                        ps = ppsum.tile([128, 512], F32, tag="p")
                        for cc in range(CC):
                            nc.tensor.matmul(
                                ps,
                                wk_sb[:, cc, ic * 128 : (ic + 1) * 128],
                                patT[:, cc, mj * 512 : (mj + 1) * 512],
                                start=(cc == 0),
                                stop=(cc == CC - 1),
                            )
                        sl = slice(mj * 512, (mj + 1) * 512)
                        nc.vector.tensor_copy(kTp[0:D, ic, 0, sl], ps[0:D, :])
                        nc.scalar.copy(kTp[D : 2 * D, ic, 1, sl], ps[D : 2 * D, :])
                # Q^T [inner, n]
                for ic in range(IC):
                    for nj in range(NJ):
                        ps = ppsum.tile([128, 512], F32, tag="p")
                        for cc in range(CC):
                            nc.tensor.matmul(
                                ps,
                                wq_sb[:, cc, ic * 128 : (ic + 1) * 128],
                                pixT[:, cc, nj * 512 : (nj + 1) * 512],
                                start=(cc == 0),
                                stop=(cc == CC - 1),
                            )
                        dst = qT[:, ic, nj * 512 : (nj + 1) * 512]
                        if (ic * NJ + nj) % 2 == 0:
                            nc.vector.tensor_copy(dst, ps)
                        else:
                            nc.scalar.copy(dst, ps)
                # V natural [m, inner] -> per-m-chunk [128, H, D+1] with ones col
                for mi in range(MT):
                    ps = ppsum.tile([128, 512], F32, tag="p")
                    for cc in range(CC):
                        nc.tensor.matmul(
                            ps,
                            patT[:, cc, mi * 128 : (mi + 1) * 128],
                            wv_sb[:, cc, :],
                            start=(cc == 0),
                            stop=(cc == CC - 1),
                        )
                    dst = v_sb[mi][:, :, 0:D]
                    src = ps.rearrange("p (h d) -> p h d", h=H)
                    if mi % 2 == 0:
                        nc.vector.tensor_copy(dst, src)
                    else:
                        nc.scalar.copy(dst, src)

            # ---- attention (flat software-pipelined loop) -----------------
            # Iterations t = h*MT + mi; the attn-out matmul for step t runs
            # one iteration behind the scores matmul, so the tensor engine
            # never stalls in-order behind an exp it doesn't yet need.
            outT = qkvpool.tile([128, IC, NSH], BF16, tag="outT")

            # Normalize sub-ops are queued as closures and issued one per
            # attention iteration: a ~4us DVE burst at a head boundary would
            # otherwise delay the next head's DVE exps (which the tensor
            # engine's attn-out matmuls wait on).
            norm_queue = []

            def normalize(h, o_ps):
                """o_ps rows 0:D hold unnormalized attn-out, row D the softmax
                denominator, rows D+1.. zeros. Transpose 32x32 blocks so the
                denominator row spreads across 32 partitions (reciprocal then
                costs FD=16, not FD=512), transpose back, broadcast across the
                D output partitions on GpSimd, multiply on DVE."""
                ic = h // 2
                po = (h % 2) * D

                def chain(nj):
                    rt = rpool.tile([32, 16, 32], F32, tag="rt")
                    rt_flat = rt.rearrange("p a b -> p (a b)")

                    def t1():
                        nc.vector.transpose(rt_flat, o_ps[nj][D : D + 32, :])

                    def rec():
                        nc.vector.reciprocal(rt[:, :, 0:1], rt[:, :, 0:1])

                    r2 = rpool.tile([32, 512], F32, tag="r2")
                    r64 = rpool.tile([D, 512], F32, tag="r64")

                    def t2_bcast():
                        nc.vector.transpose(r2, rt_flat)
                        nc.gpsimd.partition_broadcast(r64, r2[0:1, :], channels=D)

                    def mul():
                        nc.vector.tensor_mul(
                            outT[po : po + D, ic, nj * 512 : (nj + 1) * 512],
                            o_ps[nj][0:D, :],
                            r64,
                        )

                    return [t1, rec, t2_bcast, mul]

                a, b = chain(0), chain(1)
                norm_queue.extend([a[0], a[1], a[2], b[0], b[1], b[2], a[3], b[3]])

            with (
                tc.tile_pool(name="spsum", bufs=2, space="PSUM") as spsum,
                tc.tile_pool(name="opsum", bufs=4, space="PSUM") as opsum,
            ):
                o_ps_all = {}
                pending = []  # [(h, mi, at)] attn-out matmuls lagging 2 iters
                LAG = 2

                def drain_one():
                    ph, pmi, pat = pending.pop(0)
                    for nj in range(NJ):
                        nc.tensor.matmul(
                            o_ps_all[ph][nj],
                            v_sb[pmi][:, ph, :],
                            pat[:, nj * 512 : (nj + 1) * 512],
                            start=(pmi == 0),
                            stop=(pmi == MT - 1),
                        )
                    if pmi == MT - 1:
                        normalize(ph, o_ps_all.pop(ph))

                for t in range(H * MT):
                    h, mi = divmod(t, MT)
                    ic = h // 2
                    if mi == 0:
                        o_ps_all[h] = [
                            opsum.tile([128, 512], F32, tag="o", name=f"o{h}_{nj}")
                            for nj in range(NJ)
                        ]
                    s_ps = spsum.tile([128, NJ * 512], F32, tag="s")
                    for nj in range(NJ):
                        nc.tensor.matmul(
                            s_ps[:, nj * 512 : (nj + 1) * 512],
                            kTp[:, ic, h % 2, mi * 128 : (mi + 1) * 128],
                            qT[:, ic, nj * 512 : (nj + 1) * 512],
                            start=True,
                            stop=True,
                        )
                    at = attnpool.tile([128, NJ * 512], BF16, tag="at")
                    if mi % 2 == 0:
                        nc.scalar.activation(
                            at, s_ps, mybir.ActivationFunctionType.Exp, scale=SCALE
                        )
                    else:
                        # split the exp across ScalarE (exact) and DVE
                        # (Schraudolph bit-trick) so neither engine gates
                        # the attention pipeline
                        nc.scalar.activation(
                            at[:, 0:512],
                            s_ps[:, 0:512],
                            mybir.ActivationFunctionType.Exp,
                            scale=SCALE,
                        )
                        nc.vector.tensor_scalar(
                            at[:, 512:1024].bitcast(mybir.dt.int16),
                            s_ps[:, 512:1024],
                            EXPA,
                            EXPB,
                            mybir.AluOpType.mult,
                            mybir.AluOpType.add,
                        )
                    pending.append((h, mi, at))
                    if len(pending) > LAG:
                        drain_one()
                    if norm_queue:
                        norm_queue.pop(0)()
                while pending:
                    drain_one()
                while norm_queue:
                    norm_queue.pop(0)()

            # ---- output projection ---------------------------------------
            # bias is added by a k=1 matmul (ones stationary, bo moving) so the
            # PSUM->SBUF drain is a plain ScalarE copy off the critical DVE path.
            with tc.tile_pool(name="fpsum", bufs=3, space="PSUM") as fpsum:
                for ni in range(NT):
                    ps = fpsum.tile([128, CDIM], F32, tag="f")
                    nc.tensor.matmul(
                        ps,
                        ones_sb[0:1, 0:128],
                        bo_sb,
                        start=True,
                        stop=False,
                    )
                    for ic in range(IC):
                        nc.tensor.matmul(
                            ps,
                            outT[:, ic, ni * 128 : (ni + 1) * 128],
                            wo_sb[:, ic, :],
                            start=False,
                            stop=(ic == IC - 1),
                        )
                    st = stpool.tile([128, CDIM], F32, tag="st")
                    if ni % 2 == 0:
                        nc.scalar.copy(st[0:64, :], ps[0:64, :])
                        nc.vector.tensor_copy(st[64:128, :], ps[64:128, :])
                    else:
                        nc.vector.tensor_copy(st[0:64, :], ps[0:64, :])
                        nc.scalar.copy(st[64:128, :], ps[64:128, :])
                    base = ni * 128
                    nc.sync.dma_start(out[base : base + 64, :], st[0:64, :])
                    nc.sync.dma_start(out[base + 64 : base + 128, :], st[64:128, :])

    nc.finalize()
    return nc


def make_in_maps(pixel_embed, patch_embed, Wq, Wk, Wv, Wo, bo):
    bf = ml_dtypes.bfloat16
    pixel_embed = np.asarray(pixel_embed, dtype=np.float32)
    patch_embed = np.asarray(patch_embed, dtype=np.float32)
    wq = np.asarray(Wq, dtype=np.float32).astype(bf)
    wk = np.asarray(Wk, dtype=np.float32).astype(bf)
    wv = np.asarray(Wv, dtype=np.float32).astype(bf)
    wo = np.asarray(Wo, dtype=np.float32).astype(bf)
    bo = np.asarray(bo, dtype=np.float32).astype(bf)

    in_maps = []
    for core in range(N_CORES):
        bi, half = divmod(core, 2)
        px = pixel_embed[bi, half * NSH : (half + 1) * NSH, :]  # [NSH, CDIM]
        pa = patch_embed[bi]  # [M, CDIM]
        in_maps.append(
            {
                "pixelT": px.T.astype(bf),
                "patchT": pa.T.astype(bf),
                "wq": wq,
                "wk": wk,
                "wv": wv,
                "wo": wo,
                "bo": bo,
            }
        )
    return in_maps


def gather_out(results):
    out = np.empty((B, N, CDIM), np.float32)
    for core in range(N_CORES):
        bi, half = divmod(core, 2)
        out[bi, half * NSH : (half + 1) * NSH, :] = results[core]["out"]
    return out


_NC_CACHE = {}


def kernel(pixel_embed, patch_embed, Wq, Wk, Wv, Wo, bo, **kw):
    if "nc" not in _NC_CACHE:
        _NC_CACHE["nc"] = build_nc()
    nc = _NC_CACHE["nc"]
    in_maps = make_in_maps(pixel_embed, patch_embed, Wq, Wk, Wv, Wo, bo)
    res = run_bass_kernel_spmd(nc, in_maps, core_ids=list(range(N_CORES)), **kw)
    out = gather_out(res.results)
    if kw.get("trace"):
        return out, res
    return out

